# revision 41
# baseline (speedup 1.0000x reference)
"""Trainium2 Bass kernel for BailingMoeV2.5 linear attention layer.

Sharding: 8 cores = 2 batches x 4 head-groups. Core c handles batch c//4,
heads 4*(c%4) .. +4 (of 16). Each core computes its head-slice of
qkv/g projections, chunked ALiBi-decayed linear attention, group-RMSNorm,
sigmoid gate, and a partial dense output (its 512 rows of w_dense).
Host sums the 4 partial outputs per batch.

All matmuls in fp16 (1 cyc/row on PE, 10-bit mantissa), fp32 PSUM
accumulation, fp32 recurrent state master.

Layout strategy per core:
  - hiddenT (fp16, [d_in, s]) host-pre-transposed; projections of q,k,v
    token-major (hiddenT tiles stationary), g head-dim-major (W_g stationary).
  - q,k norm+rope in token-major (free-dim reductions), then PE-transposed
    per chunk into head-dim-major for attention matmuls.
  - attention output oT accumulates [e, i] in PSUM (intra + inter), group
    norm via PE ones-reduction + PE broadcast, gate in head-dim-major.
  - dense: ogT (fp16) stationary, w_dense moving -> token-major partial out.
"""

import math
from contextlib import ExitStack

import numpy as np

import concourse.mybir as mybir
import concourse.tile as tile
from concourse import bacc
from concourse.bass_utils import run_bass_kernel_spmd

dt = mybir.dt
F32 = dt.float32
F16 = dt.float16
AL = mybir.AluOpType
AF = mybir.ActivationFunctionType

# static model config
NH, HD, HID = 16, 128, 2048
ROT, HALF = 64, 32
EPS = 1e-6
THETA = 10000.0
LAYER_IDX, N_LAYERS = 1, 32
B, S = 2, 2048

DEBUG_TAPS = False
TAP_SET = ("q", "k", "v", "sg", "og", "st")

# kernel tiling config
NCORES = 8
NHL = 4            # heads per core
CH = 256           # internal chunk length (exact algebraic regrouping)
BLK = 512          # tokens per projection block
NBLK = S // BLK    # 4
SUBS = BLK // 128  # 4 s-subtiles per block
KT = HID // 128    # 16 d_in tiles
DOUT = NHL * HD    # 512 per tensor (q,k,v,g)


def _base_slopes(n):
    start = 2 ** (-(2 ** (-(math.log2(n) - 3))))
    return [start * (start ** i) for i in range(n)]


_SLOPE_ALL = -np.array(_base_slopes(NH), dtype=np.float64) * (
    1.0 - (LAYER_IDX - 1) / (N_LAYERS - 1) + 1e-5
)  # [NH] negative log-decay


def _build_module():
    nc = bacc.Bacc("TRN2", target_bir_lowering=False, debug=False,
                   num_devices=NCORES)

    f16in = lambda name, shape: nc.dram_tensor(
        name, shape, F16, kind="ExternalInput").ap()
    f32in = lambda name, shape: nc.dram_tensor(
        name, shape, F32, kind="ExternalInput").ap()

    d = {
        "hT": f16in("hT", [HID, S]),
        "wqkvg": f16in("wqkvg", [128, KT, 4 * DOUT]),
        "wd": f16in("wd", [128, NHL, HID]),
        "costab": f16in("costab", [128, S // 128, HALF]),
        "sintab": f16in("sintab", [128, S // 128, HALF]),
        "qnw": f16in("qnw", [128, HD]),
        "knw": f16in("knw", [128, HD]),
        "maskt": f16in("maskt", [128, NHL, 2, CH]),
        "identm": f16in("identm", [128, 128]),
        "onec": f16in("onec", [128, 1]),
        "oner": f16in("oner", [1, 128]),
        "qdecT": f16in("qdecT", [128, NHL, CH]),
        "kdec": f32in("kdec", [128, NHL, 2]),
        "lamc": f32in("lamc", [128, NHL]),
        "st0": f32in("st0", [128, NHL, HD]),
        "outp": nc.dram_tensor("outp", [S, HID], F16,
                               kind="ExternalOutput").ap(),
    }
    if DEBUG_TAPS:
        for nm, shape, dtp in [("dbg_q", [128, SUBS, DOUT], F16),
                               ("dbg_k", [128, SUBS, DOUT], F16),
                               ("dbg_v", [128, SUBS, DOUT], F16),
                               ("dbg_og", [128, NHL, BLK], F16),
                               ("dbg_st", [128, NHL, HD], F32),
                               ("dbg_sg", [128, NHL, BLK], F16)]:
            d[nm] = nc.dram_tensor(nm, shape, dtp,
                                   kind="ExternalOutput").ap()

    with tile.TileContext(nc) as tc, ExitStack() as ctx, \
            nc.allow_low_precision(reason="fp16 operands, fp32 accumulate"):
        _body(nc, tc, ctx, d)

    nc.compile()
    return nc


def _body(nc, tc, ctx, d):
    P = 128

    pool = lambda name, bufs: ctx.enter_context(
        tc.tile_pool(name=name, bufs=bufs))
    const = pool("const", 1)      # tables, masks, state, identity (~12k)
    wpool = pool("wpool", 1)      # 48k: resident weights (fp16)
    htpool = pool("ht", 34)       # 34k: hiddenT tiles, 2 blocks in flight
    qkvblk = pool("qkvblk", 8)    # 24k: q/k/v token-major, 2 blocks in flight
    sigp = pool("sigp", 2)        # 4k: sigmoid(g) head-dim-major (fp16)
    sqscp = pool("sqsc", 2)       # 2k: sumsq squares scratch (fp16)
    ropep = pool("ropep", 2)      # rope m1..m4 (fp16)
    ssp = pool("ssp", 8)          # ~0.5k: sumsq/rstd chains (fp32)
    trp = pool("trp", 2)          # 6k: qT/qdT/kT chunk tiles (fp16)
    stp = pool("stp", 3)          # 1.5k: masked scoresT (fp16)
    smallp = pool("smallp", 3)    # kdec-scaled v (fp16)
    stcp = pool("stcp", 2)        # 2k: fp16 state snapshot
    sqp = pool("sqp", 4)          # o squares + oS evictions
    rsp = pool("rsp", 2)          # gn rstd rows + broadcast tiles
    ogp = pool("ogp", 3)          # 12k: ogT fp16, 3 generations
    outsp = pool("outs", 3)       # 6k: dense output staging (fp32)

    psum = ctx.enter_context(tc.tile_pool(name="ps", bufs=8, space="PSUM"))
    psn = [0]

    def ps_tile(shape, dtype=F32):
        psn[0] += 1
        return psum.tile(shape, dtype, tag="ps", name=f"ps{psn[0]}")

    def load(pl, shape, dtype, name):
        t = pl.tile(shape, dtype, tag=name, name=name)
        nc.sync.dma_start(t[:], d[name])
        return t

    # per-k weight tiles + block-0 hT tiles, DMA-interleaved so the first
    # projection matmuls can start early instead of after the full 12MB
    w_tiles = []
    ht0_tiles = []
    for k in range(KT):
        wt = wpool.tile([P, 4 * DOUT], F16, tag=f"w{k}", name=f"w{k}")
        nc.sync.dma_start(wt[:], d["wqkvg"][:, k, :])
        w_tiles.append(wt)
        t = htpool.tile([P, BLK], F16, tag="ht")
        nc.sync.dma_start(t[:], d["hT"][k * 128:(k + 1) * 128, 0:BLK])
        ht0_tiles.append(t)
    wd_t = load(wpool, [P, NHL, HID], F16, "wd")
    cos_t = load(const, [P, S // 128, HALF], F16, "costab")
    sin_t = load(const, [P, S // 128, HALF], F16, "sintab")
    qnw_t = load(const, [P, HD], F16, "qnw")
    knw_t = load(const, [P, HD], F16, "knw")
    mask_t = load(const, [P, NHL, 2, CH], F16, "maskt")
    qdecT_t = load(const, [P, NHL, CH], F16, "qdecT")
    kdec_t = load(const, [P, NHL, 2], F32, "kdec")
    lamc_t = load(const, [P, NHL], F32, "lamc")
    state_t = load(const, [P, NHL, HD], F32, "st0")
    ident = load(const, [P, P], F16, "identm")
    ones_col = load(const, [P, 1], F16, "onec")
    ones_row = load(const, [1, P], F16, "oner")

    def emit_ht(nn):
        ts = []
        for k in range(KT):
            t = htpool.tile([P, BLK], F16, tag="ht")
            nc.sync.dma_start(t[:], d["hT"][k * 128:(k + 1) * 128,
                                            nn * BLK:(nn + 1) * BLK])
            ts.append(t)
        return ts

    def make_proj(ht_tiles):
        """Allocate a block's q/k/v tiles; return 12 emit-closures (one PSUM
        accumulation group each: 16 matmuls + eviction + q/k sumsq)."""
        q_blk = [qkvblk.tile([P, DOUT], F16, tag="blk0", name="qb") for _ in range(SUBS)]
        k_blk = [qkvblk.tile([P, DOUT], F16, tag="blk1", name="kb") for _ in range(SUBS)]
        v_blk = [qkvblk.tile([P, DOUT], F16, tag="blk2", name="vb") for _ in range(SUBS)]
        ss_l = [ssp.tile([P, 8], F32, tag="ss", name="ss") for _ in range(SUBS)]
        groups = []
        for sub in range(SUBS):
            for ti, dest in enumerate((q_blk, k_blk, v_blk)):
                def grp(sub=sub, ti=ti, dest=dest, ss_t=ss_l[sub],
                        ht=ht_tiles):
                    ps = ps_tile([P, DOUT])
                    for k in range(KT):
                        nc.tensor.matmul(
                            ps[:], ht[k][:, sub * 128:(sub + 1) * 128],
                            w_tiles[k][:, ti * DOUT:(ti + 1) * DOUT],
                            start=(k == 0), stop=(k == KT - 1))
                    sb = dest[sub]
                    nc.scalar.copy(sb[:], ps[:])
                    if ti < 2:
                        sqs = sqscp.tile([P, DOUT], F16, tag="sqscratch")
                        for h in range(NHL):
                            nc.vector.tensor_mul(
                                sqs[:, h * HD:(h + 1) * HD],
                                sb[:, h * HD:(h + 1) * HD],
                                sb[:, h * HD:(h + 1) * HD])
                            nc.vector.tensor_reduce(
                                ss_t[:, ti * 4 + h:ti * 4 + h + 1],
                                sqs[:, h * HD:(h + 1) * HD],
                                mybir.AxisListType.X, AL.add)
                groups.append(grp)
        return groups, (q_blk, k_blk, v_blk, ss_l)

    def emit_norm_rope(n, blk_state):
        q_blk, k_blk, v_blk, ss_l = blk_state
        for sub in range(SUBS):
            gs = n * SUBS + sub
            rstd_t = ssp.tile([P, 8], F32, tag="rstd")
            nc.vector.tensor_scalar(rstd_t[:], ss_l[sub][:], 1.0 / HD, EPS,
                                    AL.mult, AL.add)
            nc.vector.reciprocal(rstd_t[:], rstd_t[:])
            nc.scalar.activation(rstd_t[:], rstd_t[:], AF.Sqrt)
            for ti, (blk, nw_t) in enumerate(((q_blk, qnw_t), (k_blk, knw_t))):
                x = blk[sub]
                for h in range(NHL):
                    nc.vector.scalar_tensor_tensor(
                        x[:, h * HD:(h + 1) * HD], x[:, h * HD:(h + 1) * HD],
                        rstd_t[:, ti * 4 + h:ti * 4 + h + 1], nw_t[:],
                        AL.mult, AL.mult)
                x3 = x.rearrange("p (h d) -> p h d", h=NHL)
                x1, x2 = x3[:, :, 0:HALF], x3[:, :, HALF:ROT]
                cosb = cos_t[:, gs, :].unsqueeze(1).to_broadcast(
                    (P, NHL, HALF))
                sinb = sin_t[:, gs, :].unsqueeze(1).to_broadcast(
                    (P, NHL, HALF))
                m1 = ropep.tile([P, NHL, HALF], F16, tag="m1")
                m2 = ropep.tile([P, NHL, HALF], F16, tag="m2")
                m3 = ropep.tile([P, NHL, HALF], F16, tag="m3")
                m4 = ropep.tile([P, NHL, HALF], F16, tag="m4")
                nc.vector.tensor_mul(m1[:], x1, cosb)
                nc.vector.tensor_mul(m2[:], x2, sinb)
                nc.vector.tensor_mul(m3[:], x2, cosb)
                nc.vector.tensor_mul(m4[:], x1, sinb)
                nc.vector.tensor_sub(x1, m1[:], m2[:])
                nc.vector.tensor_add(x2, m3[:], m4[:])

    def dense_group(nn, ogT, sub, dm):
        ps = ps_tile([P, BLK])
        for kk in range(NHL):
            nc.tensor.matmul(
                ps[:], ogT[:, kk, sub * 128:(sub + 1) * 128],
                wd_t[:, kk, dm * BLK:(dm + 1) * BLK],
                start=(kk == 0), stop=(kk == NHL - 1))
        osb = outsp.tile([P, BLK], F16, tag="osb")
        nc.scalar.copy(osb[:], ps[:])
        eng = nc.sync if dm % 2 == 0 else nc.scalar
        eng.dma_start(
            d["outp"][nn * BLK + sub * 128:nn * BLK + (sub + 1) * 128,
                      dm * BLK:(dm + 1) * BLK], osb[:])

    # prologue: block 0 projections emitted directly
    groups0, cur = make_proj(ht0_tiles)
    for g in groups0:
        g()
    ht_cur = ht0_tiles

    filler = []

    def drain(k):
        for _ in range(min(k, len(filler))):
            filler.pop(0)()

    for n in range(NBLK):
        q_blk, k_blk, v_blk, ss_l = cur
        drain(len(filler))  # leftover dense from previous block

        emit_norm_rope(n, cur)

        # g projection emitted lazily (interleaved into attention)
        sig_blk = sigp.tile([P, NHL, BLK], F16, tag="sig")

        def g_proj_group(mg, ht=ht_cur, sig=sig_blk):
            ps = ps_tile([P, BLK])
            for k in range(KT):
                nc.tensor.matmul(
                    ps[:],
                    w_tiles[k][:, 3 * DOUT + mg * 128:3 * DOUT + (mg + 1) * 128],
                    ht[k][:], start=(k == 0), stop=(k == KT - 1))
            nc.scalar.activation(sig[:, mg, :], ps[:], AF.Sigmoid)

        filler.extend([lambda mg=mg: g_proj_group(mg) for mg in range(NHL)])

        # enqueue next block's hT loads + projection groups as filler
        if n + 1 < NBLK:
            ht_nxt = emit_ht(n + 1)
            groups, nxt = make_proj(ht_nxt)
            filler.extend(groups)
        else:
            nxt = None
        drain(2)

        if DEBUG_TAPS and n == 0:
            for sub in range(SUBS):
                if "q" in TAP_SET:
                    nc.sync.dma_start(d["dbg_q"][:, sub, :], q_blk[sub][:])
                if "k" in TAP_SET:
                    nc.sync.dma_start(d["dbg_k"][:, sub, :], k_blk[sub][:])
                if "v" in TAP_SET:
                    nc.sync.dma_start(d["dbg_v"][:, sub, :], v_blk[sub][:])

        # ---- attention: 2 chunks of 256, phase-wise over heads ----
        ogT_blk = ogp.tile([P, NHL, BLK], F16, tag="ogT")
        for cc in range(2):
            subs = (2 * cc, 2 * cc + 1)

            # fp16 snapshot of state (pre-update) + decay-scaled copies,
            # batched across heads via free-dim broadcast of the decay cols
            st_c = stcp.tile([P, NHL, HD], F16, tag="stc")
            nc.vector.tensor_copy(st_c[:], state_t[:])
            vd_c = []
            for si, sub in enumerate(subs):
                vd = smallp.tile([P, NHL, HD], F16, tag="vd")
                nc.vector.tensor_mul(
                    vd[:], v_blk[sub][:].rearrange("p (h e) -> p h e", h=NHL),
                    kdec_t[:, :, si].unsqueeze(2).to_broadcast((P, NHL, HD)))
                vd_c.append(vd)

            # transposes: per (tensor, si) 4 head-transposes share one PSUM
            # tile and one [128, 512] eviction
            qT_all = trp.tile([P, NHL, CH], F16, tag="qT")
            qdT_all = trp.tile([P, NHL, CH], F16, tag="qdT")
            kT_all = trp.tile([P, NHL, CH], F16, tag="kT")
            for si, sub in enumerate(subs):
                sl = slice(si * 128, (si + 1) * 128)
                pt = ps_tile([P, NHL * P], F16)
                for h in range(NHL):
                    nc.tensor.transpose(
                        pt[:, h * 128:(h + 1) * 128],
                        q_blk[sub][:, h * HD:(h + 1) * HD], ident[:])
                nc.scalar.copy(qT_all[:, :, sl],
                               pt[:].rearrange("p (h x) -> p h x", h=NHL))
                drain(1)
                pt3 = ps_tile([P, NHL * P], F16)
                for h in range(NHL):
                    nc.tensor.transpose(
                        pt3[:, h * 128:(h + 1) * 128],
                        k_blk[sub][:, h * HD:(h + 1) * HD], ident[:])
                nc.scalar.copy(kT_all[:, :, sl],
                               pt3[:].rearrange("p (h x) -> p h x", h=NHL))
                drain(1)
            nc.vector.tensor_mul(qdT_all[:], qT_all[:], qdecT_t[:])
            qT = [qT_all[:, h, :] for h in range(NHL)]
            qdT = [qdT_all[:, h, :] for h in range(NHL)]
            kT = [kT_all[:, h, :] for h in range(NHL)]

            # scores for all heads, masked
            sT = []
            for h in range(NHL):
                sTh = []
                for si in range(2):
                    ps = ps_tile([P, CH])
                    nc.tensor.matmul(ps[:], kT[h][:, si * 128:(si + 1) * 128],
                                     qT[h], start=True, stop=True)
                    st = stp.tile([P, CH], F16, tag="sT")
                    nc.vector.tensor_mul(st[:], ps[:], mask_t[:, h, si, :])
                    sTh.append(st)
                sT.append(sTh)
                if h % 2 == 1:
                    drain(1)

            # o (intra + inter) and state delta + update per head
            sq_tiles = []
            o_sb_l = []
            for h in range(NHL):
                o_ps = ps_tile([P, CH])
                for si, sub in enumerate(subs):
                    nc.tensor.matmul(
                        o_ps[:], v_blk[sub][:, h * HD:(h + 1) * HD],
                        sT[h][si][:], start=(si == 0), stop=False)
                nc.tensor.matmul(o_ps[:], st_c[:, h, :], qdT[h],
                                 start=False, stop=True)

                dl_ps = ps_tile([P, HD])
                for si, sub in enumerate(subs):
                    nc.tensor.matmul(
                        dl_ps[:], k_blk[sub][:, h * HD:(h + 1) * HD],
                        vd_c[si][:, h, :], start=(si == 0), stop=(si == 1))
                nc.vector.scalar_tensor_tensor(
                    state_t[:, h, :], state_t[:, h, :],
                    lamc_t[:, h:h + 1], dl_ps[:], AL.mult, AL.add)

                # scale 1/64 before squaring: o can reach ~1e3 for
                # weak-decay heads and o^2 would overflow fp16
                sq = sqp.tile([P, CH], F16, tag="sq")
                nc.scalar.activation(sq[:], o_ps[:], AF.Square,
                                     scale=1.0 / 64.0)
                # evict o to SBUF so the PSUM slot frees without waiting
                # for the group-norm chain
                oS = sqp.tile([P, CH], F16, tag="oS")
                nc.scalar.copy(oS[:], o_ps[:])
                sq_tiles.append(sq)
                o_sb_l.append(oS)
                if h % 2 == 1:
                    drain(1)

            # group norm rstd in [1, CH] row form: ones-reduce over the
            # 256 group dims on PE, rsqrt chain, gpsimd partition
            # broadcast, folded into the sigmoid gate
            for g in range(NHL // 2):
                gss = ps_tile([1, CH])
                for h in (2 * g, 2 * g + 1):
                    nc.tensor.matmul(gss[:], ones_col[:], sq_tiles[h][:],
                                     start=(h == 2 * g),
                                     stop=(h == 2 * g + 1))
                rrow32 = rsp.tile([1, CH], F32, tag="rrow32")
                nc.vector.tensor_scalar(rrow32[:], gss[:],
                                        4096.0 / (2 * HD), EPS,
                                        AL.mult, AL.add)
                rrow = rsp.tile([1, CH], F16, tag="rrow")
                nc.scalar.activation(rrow[:], rrow32[:], AF.Sqrt)
                bc = rsp.tile([P, CH], F16, tag="bc")
                nc.gpsimd.partition_broadcast(bc[:], rrow[:])
                # reciprocal AFTER the broadcast: [128, CH] is lane-parallel
                # on DVE while [1, CH] is 256 serial elements on one lane
                nc.vector.reciprocal(bc[:], bc[:])
                nc.vector.tensor_mul(
                    sig_blk[:, 2 * g:2 * g + 2, cc * CH:(cc + 1) * CH],
                    sig_blk[:, 2 * g:2 * g + 2, cc * CH:(cc + 1) * CH],
                    bc[:].unsqueeze(1).to_broadcast((P, 2, CH)))
                drain(1)
            for h in range(NHL):
                nc.vector.tensor_mul(
                    ogT_blk[:, h, cc * CH:(cc + 1) * CH], o_sb_l[h][:],
                    sig_blk[:, h, cc * CH:(cc + 1) * CH])
            for sub in (2 * cc, 2 * cc + 1):
                for dm in range(HID // BLK):
                    filler.append(
                        lambda nn=n, og=ogT_blk, s=sub, m=dm:
                        dense_group(nn, og, s, m))
            drain(2)

        if DEBUG_TAPS and n == 0:
            if "sg" in TAP_SET:
                nc.sync.dma_start(d["dbg_sg"], sig_blk[:])
            if "og" in TAP_SET:
                nc.sync.dma_start(d["dbg_og"], ogT_blk[:])
            if "st" in TAP_SET:
                st_dump = outsp.tile([P, NHL, HD], F32, tag="stdump")
                nc.vector.tensor_copy(st_dump[:], state_t[:])
                nc.sync.dma_start(d["dbg_st"], st_dump[:])

        cur = nxt
        ht_cur = ht_nxt if n + 1 < NBLK else None

    drain(len(filler))


_NC_CACHE = None


def _get_module():
    global _NC_CACHE
    if _NC_CACHE is None:
        _NC_CACHE = _build_module()
    return _NC_CACHE


def _host_inputs(positions, hidden_states, recurrent_state, w_qkv, w_g,
                 w_dense, q_norm_w, k_norm_w, g_norm_w):
    """Build the 8 per-core input dicts."""
    F16NP = np.float16
    positions = np.asarray(positions)
    hidden_states = np.asarray(hidden_states, dtype=np.float32)
    recurrent_state = np.asarray(recurrent_state, dtype=np.float32)
    w_qkv = np.asarray(w_qkv, dtype=np.float32)
    w_g = np.asarray(w_g, dtype=np.float32)
    w_dense = np.asarray(w_dense, dtype=np.float32)
    q_norm_w = np.asarray(q_norm_w, dtype=np.float32)
    k_norm_w = np.asarray(k_norm_w, dtype=np.float32)
    g_norm_w = np.asarray(g_norm_w, dtype=np.float32)

    # rope tables from positions: [S, HALF] -> [128, S//128, HALF]
    inv_freq = 1.0 / (THETA ** (np.arange(HALF, dtype=np.float64) / HALF))
    ang = positions.astype(np.float64)[:, None] * inv_freq[None, :]
    cos = np.cos(ang).reshape(S // 128, 128, HALF).transpose(1, 0, 2)
    sin = np.sin(ang).reshape(S // 128, 128, HALF).transpose(1, 0, 2)
    cos = np.ascontiguousarray(cos.astype(F16NP))
    sin = np.ascontiguousarray(sin.astype(F16NP))

    qnw = np.ascontiguousarray(np.tile(q_norm_w[None, :], (128, 1))
                               .astype(F16NP))
    knw = np.ascontiguousarray(np.tile(k_norm_w[None, :], (128, 1))
                               .astype(F16NP))

    i_idx = np.arange(CH, dtype=np.float64)
    j_idx = np.arange(CH, dtype=np.float64)

    in_maps = []
    for core in range(NCORES):
        b = core // 4
        h0 = (core % 4) * NHL  # global head offset
        cs, ce = h0 * HD, (h0 + NHL) * HD

        hT = np.ascontiguousarray(hidden_states[b].T.astype(F16NP))

        w_all = np.concatenate(
            [w_qkv[:, cs:ce], w_qkv[:, NH * HD + cs:NH * HD + ce],
             w_qkv[:, 2 * NH * HD + cs:2 * NH * HD + ce],
             w_g[:, cs:ce]], axis=1)  # [HID, 4*DOUT]
        wqkvg = np.ascontiguousarray(
            w_all.reshape(KT, 128, 4 * DOUT).transpose(1, 0, 2).astype(F16NP))

        wd = (w_dense[cs:ce, :] * g_norm_w[cs:ce, None]).astype(F16NP)
        wd = np.ascontiguousarray(wd.reshape(NHL, 128, HID).transpose(1, 0, 2))

        loglam = _SLOPE_ALL[h0:h0 + NHL]  # [NHL] negative
        # maskt[p, h, jsub, i] = lam^(i - (jsub*128+p)) for i >= j else 0
        mm = np.where(i_idx[None, None, :] >= j_idx[None, :, None],
                      np.exp(loglam[:, None, None]
                             * (i_idx[None, None, :] - j_idx[None, :, None])),
                      0.0)  # [NHL, j, i]
        maskt = np.ascontiguousarray(
            mm.reshape(NHL, 2, 128, CH).transpose(2, 0, 1, 3).astype(F16NP))

        qdecT = np.exp(loglam[:, None] * (i_idx[None, :] + 1.0))  # [NHL, CH]
        qdecT = np.ascontiguousarray(
            np.broadcast_to(qdecT[None, :, :], (128, NHL, CH))
            .astype(F16NP))
        kdec = np.exp(loglam[:, None] * (CH - 1.0 - j_idx[None, :]))
        kdec = np.ascontiguousarray(
            kdec.reshape(NHL, 2, 128).transpose(2, 0, 1).astype(np.float32))
        lamc = np.tile(np.exp(loglam * CH).astype(np.float32)[None, :],
                       (128, 1))
        lamc = np.ascontiguousarray(lamc)

        st0 = np.ascontiguousarray(
            recurrent_state[b, h0:h0 + NHL].transpose(1, 0, 2))  # [d, h, e]

        in_maps.append({
            "identm": np.eye(128, dtype=F16NP),
            "onec": np.ones((128, 1), F16NP),
            "oner": np.ones((1, 128), F16NP),
            "hT": hT, "wqkvg": wqkvg, "wd": wd,
            "costab": cos, "sintab": sin, "qnw": qnw, "knw": knw,
            "maskt": maskt, "qdecT": qdecT, "kdec": kdec, "lamc": lamc,
            "st0": st0.astype(np.float32),
        })
    return in_maps


def kernel(**inputs):
    nc = _get_module()
    in_maps = _host_inputs(**inputs)
    res = run_bass_kernel_spmd(nc, in_maps, core_ids=list(range(NCORES)))
    outs = [np.asarray(r["outp"], dtype=np.float32) for r in res.results]
    out = np.stack([outs[0] + outs[1] + outs[2] + outs[3],
                    outs[4] + outs[5] + outs[6] + outs[7]])
    return out.astype(np.float32)



# revision 44
# speedup vs baseline: 1.0153x; 1.0153x over previous
"""Trainium2 Bass kernel for BailingMoeV2.5 linear attention layer.

Sharding: 8 cores = 2 batches x 4 head-groups. Core c handles batch c//4,
heads 4*(c%4) .. +4 (of 16). Each core computes its head-slice of
qkv/g projections, chunked ALiBi-decayed linear attention, group-RMSNorm,
sigmoid gate, and a partial dense output (its 512 rows of w_dense).
Host sums the 4 partial outputs per batch.

All matmuls in fp16 (1 cyc/row on PE, 10-bit mantissa), fp32 PSUM
accumulation, fp32 recurrent state master.

Layout strategy per core:
  - hiddenT (fp16, [d_in, s]) host-pre-transposed; projections of q,k,v
    token-major (hiddenT tiles stationary), g head-dim-major (W_g stationary).
  - q,k norm+rope in token-major (free-dim reductions), then PE-transposed
    per chunk into head-dim-major for attention matmuls.
  - attention output oT accumulates [e, i] in PSUM (intra + inter), group
    norm via PE ones-reduction + PE broadcast, gate in head-dim-major.
  - dense: ogT (fp16) stationary, w_dense moving -> token-major partial out.
"""

import math
from contextlib import ExitStack

import numpy as np

import concourse.mybir as mybir
import concourse.tile as tile
from concourse import bacc
from concourse.bass_utils import run_bass_kernel_spmd

dt = mybir.dt
F32 = dt.float32
F16 = dt.float16
AL = mybir.AluOpType
AF = mybir.ActivationFunctionType

# static model config
NH, HD, HID = 16, 128, 2048
ROT, HALF = 64, 32
EPS = 1e-6
THETA = 10000.0
LAYER_IDX, N_LAYERS = 1, 32
B, S = 2, 2048

DEBUG_TAPS = False
TAP_SET = ("q", "k", "v", "sg", "og", "st")

# kernel tiling config
NCORES = 8
NHL = 4            # heads per core
CH = 256           # internal chunk length (exact algebraic regrouping)
BLK = 512          # tokens per projection block
NBLK = S // BLK    # 4
SUBS = BLK // 128  # 4 s-subtiles per block
KT = HID // 128    # 16 d_in tiles
DOUT = NHL * HD    # 512 per tensor (q,k,v,g)


def _base_slopes(n):
    start = 2 ** (-(2 ** (-(math.log2(n) - 3))))
    return [start * (start ** i) for i in range(n)]


_SLOPE_ALL = -np.array(_base_slopes(NH), dtype=np.float64) * (
    1.0 - (LAYER_IDX - 1) / (N_LAYERS - 1) + 1e-5
)  # [NH] negative log-decay


def _build_module():
    nc = bacc.Bacc("TRN2", target_bir_lowering=False, debug=False,
                   num_devices=NCORES)

    f16in = lambda name, shape: nc.dram_tensor(
        name, shape, F16, kind="ExternalInput").ap()
    f32in = lambda name, shape: nc.dram_tensor(
        name, shape, F32, kind="ExternalInput").ap()

    d = {
        "hT": f16in("hT", [HID, S]),
        "wqkvg": f16in("wqkvg", [128, KT, 4 * DOUT]),
        "wd": f16in("wd", [128, NHL, HID]),
        "costab": f16in("costab", [128, S // 128, HALF]),
        "sintab": f16in("sintab", [128, S // 128, HALF]),
        "qnw": f16in("qnw", [128, HD]),
        "knw": f16in("knw", [128, HD]),
        "maskt": f16in("maskt", [128, NHL, 2, CH]),
        "identm": f16in("identm", [128, 128]),
        "onec": f16in("onec", [128, 1]),
        "oner": f16in("oner", [1, 128]),
        "qdecT": f16in("qdecT", [128, NHL, CH]),
        "kdec": f32in("kdec", [128, NHL, 2]),
        "lamc": f32in("lamc", [128, NHL]),
        "epsc": f32in("epsc", [1, 1]),
        "st0": f32in("st0", [128, NHL, HD]),
        "outp": nc.dram_tensor("outp", [S, HID], F16,
                               kind="ExternalOutput").ap(),
    }
    if DEBUG_TAPS:
        for nm, shape, dtp in [("dbg_q", [128, SUBS, DOUT], F16),
                               ("dbg_k", [128, SUBS, DOUT], F16),
                               ("dbg_v", [128, SUBS, DOUT], F16),
                               ("dbg_og", [128, NHL, BLK], F16),
                               ("dbg_st", [128, NHL, HD], F32),
                               ("dbg_sg", [128, NHL, BLK], F16)]:
            d[nm] = nc.dram_tensor(nm, shape, dtp,
                                   kind="ExternalOutput").ap()

    with tile.TileContext(nc) as tc, ExitStack() as ctx, \
            nc.allow_low_precision(reason="fp16 operands, fp32 accumulate"):
        _body(nc, tc, ctx, d)

    nc.compile()
    return nc


def _body(nc, tc, ctx, d):
    P = 128

    pool = lambda name, bufs: ctx.enter_context(
        tc.tile_pool(name=name, bufs=bufs))
    const = pool("const", 1)      # tables, masks, state, identity (~12k)
    wpool = pool("wpool", 1)      # 48k: resident weights (fp16)
    htpool = pool("ht", 34)       # 34k: hiddenT tiles, 2 blocks in flight
    qkvblk = pool("qkvblk", 8)    # 24k: q/k/v token-major, 2 blocks in flight
    sigp = pool("sigp", 2)        # 4k: sigmoid(g) head-dim-major (fp16)
    sqscp = pool("sqsc", 2)       # 2k: sumsq squares scratch (fp16)
    ropep = pool("ropep", 2)      # rope m1..m4 (fp16)
    ssp = pool("ssp", 8)          # ~0.5k: sumsq/rstd chains (fp32)
    trp = pool("trp", 2)          # 6k: qT/qdT/kT chunk tiles (fp16)
    stp = pool("stp", 3)          # 1.5k: masked scoresT (fp16)
    smallp = pool("smallp", 3)    # kdec-scaled v (fp16)
    stcp = pool("stcp", 2)        # 2k: fp16 state snapshot
    sqp = pool("sqp", 4)          # o squares + oS evictions
    rsp = pool("rsp", 2)          # gn rstd rows + broadcast tiles
    ogp = pool("ogp", 3)          # 12k: ogT fp16, 3 generations
    outsp = pool("outs", 3)       # 6k: dense output staging (fp32)

    psum = ctx.enter_context(tc.tile_pool(name="ps", bufs=8, space="PSUM"))
    psn = [0]

    def ps_tile(shape, dtype=F32):
        psn[0] += 1
        return psum.tile(shape, dtype, tag="ps", name=f"ps{psn[0]}")

    def load(pl, shape, dtype, name):
        t = pl.tile(shape, dtype, tag=name, name=name)
        nc.sync.dma_start(t[:], d[name])
        return t

    # per-k weight tiles + block-0 hT tiles, DMA-interleaved so the first
    # projection matmuls can start early instead of after the full 12MB
    w_tiles = []
    ht0_tiles = []
    for k in range(KT):
        wt = wpool.tile([P, 4 * DOUT], F16, tag=f"w{k}", name=f"w{k}")
        nc.sync.dma_start(wt[:], d["wqkvg"][:, k, :])
        w_tiles.append(wt)
        t = htpool.tile([P, BLK], F16, tag="ht")
        nc.sync.dma_start(t[:], d["hT"][k * 128:(k + 1) * 128, 0:BLK])
        ht0_tiles.append(t)
    wd_t = load(wpool, [P, NHL, HID], F16, "wd")
    cos_t = load(const, [P, S // 128, HALF], F16, "costab")
    sin_t = load(const, [P, S // 128, HALF], F16, "sintab")
    qnw_t = load(const, [P, HD], F16, "qnw")
    knw_t = load(const, [P, HD], F16, "knw")
    mask_t = load(const, [P, NHL, 2, CH], F16, "maskt")
    qdecT_t = load(const, [P, NHL, CH], F16, "qdecT")
    kdec_t = load(const, [P, NHL, 2], F32, "kdec")
    lamc_t = load(const, [P, NHL], F32, "lamc")
    eps_t = load(const, [1, 1], F32, "epsc")
    state_t = load(const, [P, NHL, HD], F32, "st0")
    ident = load(const, [P, P], F16, "identm")
    ones_col = load(const, [P, 1], F16, "onec")
    ones_row = load(const, [1, P], F16, "oner")

    def emit_ht(nn):
        ts = []
        for k in range(KT):
            t = htpool.tile([P, BLK], F16, tag="ht")
            nc.sync.dma_start(t[:], d["hT"][k * 128:(k + 1) * 128,
                                            nn * BLK:(nn + 1) * BLK])
            ts.append(t)
        return ts

    def make_proj(ht_tiles):
        """Allocate a block's q/k/v tiles; return 12 emit-closures (one PSUM
        accumulation group each: 16 matmuls + eviction + q/k sumsq)."""
        q_blk = [qkvblk.tile([P, DOUT], F16, tag="blk0", name="qb") for _ in range(SUBS)]
        k_blk = [qkvblk.tile([P, DOUT], F16, tag="blk1", name="kb") for _ in range(SUBS)]
        v_blk = [qkvblk.tile([P, DOUT], F16, tag="blk2", name="vb") for _ in range(SUBS)]
        ss_l = [ssp.tile([P, 8], F32, tag="ss", name="ss") for _ in range(SUBS)]
        groups = []
        for sub in range(SUBS):
            for ti, dest in enumerate((q_blk, k_blk, v_blk)):
                def grp(sub=sub, ti=ti, dest=dest, ss_t=ss_l[sub],
                        ht=ht_tiles):
                    ps = ps_tile([P, DOUT])
                    for k in range(KT):
                        nc.tensor.matmul(
                            ps[:], ht[k][:, sub * 128:(sub + 1) * 128],
                            w_tiles[k][:, ti * DOUT:(ti + 1) * DOUT],
                            start=(k == 0), stop=(k == KT - 1))
                    sb = dest[sub]
                    nc.scalar.copy(sb[:], ps[:])
                    if ti < 2:
                        sqs = sqscp.tile([P, DOUT], F16, tag="sqscratch")
                        for h in range(NHL):
                            nc.vector.tensor_mul(
                                sqs[:, h * HD:(h + 1) * HD],
                                sb[:, h * HD:(h + 1) * HD],
                                sb[:, h * HD:(h + 1) * HD])
                            nc.vector.tensor_reduce(
                                ss_t[:, ti * 4 + h:ti * 4 + h + 1],
                                sqs[:, h * HD:(h + 1) * HD],
                                mybir.AxisListType.X, AL.add)
                groups.append(grp)
        return groups, (q_blk, k_blk, v_blk, ss_l)

    def emit_norm_rope(n, blk_state):
        q_blk, k_blk, v_blk, ss_l = blk_state
        for sub in range(SUBS):
            gs = n * SUBS + sub
            rstd_t = ssp.tile([P, 8], F32, tag="rstd")
            nc.vector.tensor_scalar(rstd_t[:], ss_l[sub][:], 1.0 / HD, EPS,
                                    AL.mult, AL.add)
            nc.vector.reciprocal(rstd_t[:], rstd_t[:])
            nc.scalar.activation(rstd_t[:], rstd_t[:], AF.Sqrt)
            for ti, (blk, nw_t) in enumerate(((q_blk, qnw_t), (k_blk, knw_t))):
                x = blk[sub]
                for h in range(NHL):
                    nc.vector.scalar_tensor_tensor(
                        x[:, h * HD:(h + 1) * HD], x[:, h * HD:(h + 1) * HD],
                        rstd_t[:, ti * 4 + h:ti * 4 + h + 1], nw_t[:],
                        AL.mult, AL.mult)
                x3 = x.rearrange("p (h d) -> p h d", h=NHL)
                x1, x2 = x3[:, :, 0:HALF], x3[:, :, HALF:ROT]
                cosb = cos_t[:, gs, :].unsqueeze(1).to_broadcast(
                    (P, NHL, HALF))
                sinb = sin_t[:, gs, :].unsqueeze(1).to_broadcast(
                    (P, NHL, HALF))
                m1 = ropep.tile([P, NHL, HALF], F16, tag="m1")
                m2 = ropep.tile([P, NHL, HALF], F16, tag="m2")
                m3 = ropep.tile([P, NHL, HALF], F16, tag="m3")
                m4 = ropep.tile([P, NHL, HALF], F16, tag="m4")
                nc.vector.tensor_mul(m1[:], x1, cosb)
                nc.vector.tensor_mul(m2[:], x2, sinb)
                nc.vector.tensor_mul(m3[:], x2, cosb)
                nc.vector.tensor_mul(m4[:], x1, sinb)
                nc.vector.tensor_sub(x1, m1[:], m2[:])
                nc.vector.tensor_add(x2, m3[:], m4[:])

    def dense_group(nn, ogT, sub, dm):
        ps = ps_tile([P, BLK])
        for kk in range(NHL):
            nc.tensor.matmul(
                ps[:], ogT[:, kk, sub * 128:(sub + 1) * 128],
                wd_t[:, kk, dm * BLK:(dm + 1) * BLK],
                start=(kk == 0), stop=(kk == NHL - 1))
        osb = outsp.tile([P, BLK], F16, tag="osb")
        nc.scalar.copy(osb[:], ps[:])
        eng = nc.sync if dm % 2 == 0 else nc.scalar
        eng.dma_start(
            d["outp"][nn * BLK + sub * 128:nn * BLK + (sub + 1) * 128,
                      dm * BLK:(dm + 1) * BLK], osb[:])

    # prologue: block 0 projections emitted directly
    groups0, cur = make_proj(ht0_tiles)
    for g in groups0:
        g()
    ht_cur = ht0_tiles

    filler = []

    def drain(k):
        for _ in range(min(k, len(filler))):
            filler.pop(0)()

    for n in range(NBLK):
        q_blk, k_blk, v_blk, ss_l = cur
        drain(len(filler))  # leftover dense from previous block

        emit_norm_rope(n, cur)

        # g projection emitted lazily (interleaved into attention)
        sig_blk = sigp.tile([P, NHL, BLK], F16, tag="sig")

        def g_proj_group(mg, ht=ht_cur, sig=sig_blk):
            ps = ps_tile([P, BLK])
            for k in range(KT):
                nc.tensor.matmul(
                    ps[:],
                    w_tiles[k][:, 3 * DOUT + mg * 128:3 * DOUT + (mg + 1) * 128],
                    ht[k][:], start=(k == 0), stop=(k == KT - 1))
            nc.scalar.activation(sig[:, mg, :], ps[:], AF.Sigmoid)

        filler.extend([lambda mg=mg: g_proj_group(mg) for mg in range(NHL)])

        # enqueue next block's hT loads + projection groups as filler
        if n + 1 < NBLK:
            ht_nxt = emit_ht(n + 1)
            groups, nxt = make_proj(ht_nxt)
            filler.extend(groups)
        else:
            nxt = None
        drain(2)

        if DEBUG_TAPS and n == 0:
            for sub in range(SUBS):
                if "q" in TAP_SET:
                    nc.sync.dma_start(d["dbg_q"][:, sub, :], q_blk[sub][:])
                if "k" in TAP_SET:
                    nc.sync.dma_start(d["dbg_k"][:, sub, :], k_blk[sub][:])
                if "v" in TAP_SET:
                    nc.sync.dma_start(d["dbg_v"][:, sub, :], v_blk[sub][:])

        # ---- attention: 2 chunks of 256, phase-wise over heads ----
        ogT_blk = ogp.tile([P, NHL, BLK], F16, tag="ogT")
        for cc in range(2):
            subs = (2 * cc, 2 * cc + 1)

            # fp16 snapshot of state (pre-update) + decay-scaled copies,
            # batched across heads via free-dim broadcast of the decay cols
            st_c = stcp.tile([P, NHL, HD], F16, tag="stc")
            nc.vector.tensor_copy(st_c[:], state_t[:])
            vd_c = []
            for si, sub in enumerate(subs):
                vd = smallp.tile([P, NHL, HD], F16, tag="vd")
                nc.vector.tensor_mul(
                    vd[:], v_blk[sub][:].rearrange("p (h e) -> p h e", h=NHL),
                    kdec_t[:, :, si].unsqueeze(2).to_broadcast((P, NHL, HD)))
                vd_c.append(vd)

            # transposes: per (tensor, si) 4 head-transposes share one PSUM
            # tile and one [128, 512] eviction
            qT_all = trp.tile([P, NHL, CH], F16, tag="qT")
            qdT_all = trp.tile([P, NHL, CH], F16, tag="qdT")
            kT_all = trp.tile([P, NHL, CH], F16, tag="kT")
            for si, sub in enumerate(subs):
                sl = slice(si * 128, (si + 1) * 128)
                pt = ps_tile([P, NHL * P], F16)
                for h in range(NHL):
                    nc.tensor.transpose(
                        pt[:, h * 128:(h + 1) * 128],
                        q_blk[sub][:, h * HD:(h + 1) * HD], ident[:])
                nc.scalar.copy(qT_all[:, :, sl],
                               pt[:].rearrange("p (h x) -> p h x", h=NHL))
                drain(1)
                pt3 = ps_tile([P, NHL * P], F16)
                for h in range(NHL):
                    nc.tensor.transpose(
                        pt3[:, h * 128:(h + 1) * 128],
                        k_blk[sub][:, h * HD:(h + 1) * HD], ident[:])
                nc.scalar.copy(kT_all[:, :, sl],
                               pt3[:].rearrange("p (h x) -> p h x", h=NHL))
                drain(1)
            nc.vector.tensor_mul(qdT_all[:], qT_all[:], qdecT_t[:])
            qT = [qT_all[:, h, :] for h in range(NHL)]
            qdT = [qdT_all[:, h, :] for h in range(NHL)]
            kT = [kT_all[:, h, :] for h in range(NHL)]

            # scores for all heads, masked
            sT = []
            for h in range(NHL):
                sTh = []
                for si in range(2):
                    ps = ps_tile([P, CH])
                    nc.tensor.matmul(ps[:], kT[h][:, si * 128:(si + 1) * 128],
                                     qT[h], start=True, stop=True)
                    st = stp.tile([P, CH], F16, tag="sT")
                    nc.vector.tensor_mul(st[:], ps[:], mask_t[:, h, si, :])
                    sTh.append(st)
                sT.append(sTh)
                if h % 2 == 1:
                    drain(1)

            # o (intra + inter) and state delta + update per head
            sq_tiles = []
            o_sb_l = []
            for h in range(NHL):
                o_ps = ps_tile([P, CH])
                for si, sub in enumerate(subs):
                    nc.tensor.matmul(
                        o_ps[:], v_blk[sub][:, h * HD:(h + 1) * HD],
                        sT[h][si][:], start=(si == 0), stop=False)
                nc.tensor.matmul(o_ps[:], st_c[:, h, :], qdT[h],
                                 start=False, stop=True)

                dl_ps = ps_tile([P, HD])
                for si, sub in enumerate(subs):
                    nc.tensor.matmul(
                        dl_ps[:], k_blk[sub][:, h * HD:(h + 1) * HD],
                        vd_c[si][:, h, :], start=(si == 0), stop=(si == 1))
                nc.vector.scalar_tensor_tensor(
                    state_t[:, h, :], state_t[:, h, :],
                    lamc_t[:, h:h + 1], dl_ps[:], AL.mult, AL.add)

                # scale 1/64 before squaring: o can reach ~1e3 for
                # weak-decay heads and o^2 would overflow fp16
                sq = sqp.tile([P, CH], F16, tag="sq")
                nc.scalar.activation(sq[:], o_ps[:], AF.Square,
                                     scale=1.0 / 64.0)
                # evict o to SBUF so the PSUM slot frees without waiting
                # for the group-norm chain
                oS = sqp.tile([P, CH], F16, tag="oS")
                nc.scalar.copy(oS[:], o_ps[:])
                sq_tiles.append(sq)
                o_sb_l.append(oS)
                if h % 2 == 1:
                    drain(1)

            # group norm rstd in [1, CH] row form: ones-reduce over the
            # 256 group dims on PE, rsqrt chain, gpsimd partition
            # broadcast, folded into the sigmoid gate
            for g in range(NHL // 2):
                gss = ps_tile([1, CH])
                for h in (2 * g, 2 * g + 1):
                    nc.tensor.matmul(gss[:], ones_col[:], sq_tiles[h][:],
                                     start=(h == 2 * g),
                                     stop=(h == 2 * g + 1))
                # rms row = sqrt(16*gss + eps), fused on Scalar; broadcast,
                # then ONE divide folds it into the gate (lane-parallel)
                rrow = rsp.tile([1, CH], F16, tag="rrow")
                nc.scalar.activation(rrow[:], gss[:], AF.Sqrt,
                                     bias=eps_t[:], scale=4096.0 / (2 * HD))
                bc = rsp.tile([P, CH], F16, tag="bc")
                nc.gpsimd.partition_broadcast(bc[:], rrow[:])
                nc.vector.reciprocal(bc[:], bc[:])
                nc.vector.tensor_mul(
                    sig_blk[:, 2 * g:2 * g + 2, cc * CH:(cc + 1) * CH],
                    sig_blk[:, 2 * g:2 * g + 2, cc * CH:(cc + 1) * CH],
                    bc[:].unsqueeze(1).to_broadcast((P, 2, CH)))
                drain(1)
            for h in range(NHL):
                nc.vector.tensor_mul(
                    ogT_blk[:, h, cc * CH:(cc + 1) * CH], o_sb_l[h][:],
                    sig_blk[:, h, cc * CH:(cc + 1) * CH])
            for sub in (2 * cc, 2 * cc + 1):
                for dm in range(HID // BLK):
                    filler.append(
                        lambda nn=n, og=ogT_blk, s=sub, m=dm:
                        dense_group(nn, og, s, m))
            drain(2)

        if DEBUG_TAPS and n == 0:
            if "sg" in TAP_SET:
                nc.sync.dma_start(d["dbg_sg"], sig_blk[:])
            if "og" in TAP_SET:
                nc.sync.dma_start(d["dbg_og"], ogT_blk[:])
            if "st" in TAP_SET:
                st_dump = outsp.tile([P, NHL, HD], F32, tag="stdump")
                nc.vector.tensor_copy(st_dump[:], state_t[:])
                nc.sync.dma_start(d["dbg_st"], st_dump[:])

        cur = nxt
        ht_cur = ht_nxt if n + 1 < NBLK else None

    drain(len(filler))


_NC_CACHE = None


def _get_module():
    global _NC_CACHE
    if _NC_CACHE is None:
        _NC_CACHE = _build_module()
    return _NC_CACHE


def _host_inputs(positions, hidden_states, recurrent_state, w_qkv, w_g,
                 w_dense, q_norm_w, k_norm_w, g_norm_w):
    """Build the 8 per-core input dicts."""
    F16NP = np.float16
    positions = np.asarray(positions)
    hidden_states = np.asarray(hidden_states, dtype=np.float32)
    recurrent_state = np.asarray(recurrent_state, dtype=np.float32)
    w_qkv = np.asarray(w_qkv, dtype=np.float32)
    w_g = np.asarray(w_g, dtype=np.float32)
    w_dense = np.asarray(w_dense, dtype=np.float32)
    q_norm_w = np.asarray(q_norm_w, dtype=np.float32)
    k_norm_w = np.asarray(k_norm_w, dtype=np.float32)
    g_norm_w = np.asarray(g_norm_w, dtype=np.float32)

    # rope tables from positions: [S, HALF] -> [128, S//128, HALF]
    inv_freq = 1.0 / (THETA ** (np.arange(HALF, dtype=np.float64) / HALF))
    ang = positions.astype(np.float64)[:, None] * inv_freq[None, :]
    cos = np.cos(ang).reshape(S // 128, 128, HALF).transpose(1, 0, 2)
    sin = np.sin(ang).reshape(S // 128, 128, HALF).transpose(1, 0, 2)
    cos = np.ascontiguousarray(cos.astype(F16NP))
    sin = np.ascontiguousarray(sin.astype(F16NP))

    qnw = np.ascontiguousarray(np.tile(q_norm_w[None, :], (128, 1))
                               .astype(F16NP))
    knw = np.ascontiguousarray(np.tile(k_norm_w[None, :], (128, 1))
                               .astype(F16NP))

    i_idx = np.arange(CH, dtype=np.float64)
    j_idx = np.arange(CH, dtype=np.float64)

    in_maps = []
    for core in range(NCORES):
        b = core // 4
        h0 = (core % 4) * NHL  # global head offset
        cs, ce = h0 * HD, (h0 + NHL) * HD

        hT = np.ascontiguousarray(hidden_states[b].T.astype(F16NP))

        w_all = np.concatenate(
            [w_qkv[:, cs:ce], w_qkv[:, NH * HD + cs:NH * HD + ce],
             w_qkv[:, 2 * NH * HD + cs:2 * NH * HD + ce],
             w_g[:, cs:ce]], axis=1)  # [HID, 4*DOUT]
        wqkvg = np.ascontiguousarray(
            w_all.reshape(KT, 128, 4 * DOUT).transpose(1, 0, 2).astype(F16NP))

        wd = (w_dense[cs:ce, :] * g_norm_w[cs:ce, None]).astype(F16NP)
        wd = np.ascontiguousarray(wd.reshape(NHL, 128, HID).transpose(1, 0, 2))

        loglam = _SLOPE_ALL[h0:h0 + NHL]  # [NHL] negative
        # maskt[p, h, jsub, i] = lam^(i - (jsub*128+p)) for i >= j else 0
        mm = np.where(i_idx[None, None, :] >= j_idx[None, :, None],
                      np.exp(loglam[:, None, None]
                             * (i_idx[None, None, :] - j_idx[None, :, None])),
                      0.0)  # [NHL, j, i]
        maskt = np.ascontiguousarray(
            mm.reshape(NHL, 2, 128, CH).transpose(2, 0, 1, 3).astype(F16NP))

        qdecT = np.exp(loglam[:, None] * (i_idx[None, :] + 1.0))  # [NHL, CH]
        qdecT = np.ascontiguousarray(
            np.broadcast_to(qdecT[None, :, :], (128, NHL, CH))
            .astype(F16NP))
        kdec = np.exp(loglam[:, None] * (CH - 1.0 - j_idx[None, :]))
        kdec = np.ascontiguousarray(
            kdec.reshape(NHL, 2, 128).transpose(2, 0, 1).astype(np.float32))
        lamc = np.tile(np.exp(loglam * CH).astype(np.float32)[None, :],
                       (128, 1))
        lamc = np.ascontiguousarray(lamc)

        st0 = np.ascontiguousarray(
            recurrent_state[b, h0:h0 + NHL].transpose(1, 0, 2))  # [d, h, e]

        in_maps.append({
            "identm": np.eye(128, dtype=F16NP),
            "onec": np.ones((128, 1), F16NP),
            "oner": np.ones((1, 128), F16NP),
            "hT": hT, "wqkvg": wqkvg, "wd": wd,
            "costab": cos, "sintab": sin, "qnw": qnw, "knw": knw,
            "maskt": maskt, "qdecT": qdecT, "kdec": kdec, "lamc": lamc,
            "epsc": np.full((1, 1), EPS, np.float32),
            "st0": st0.astype(np.float32),
        })
    return in_maps


def kernel(**inputs):
    nc = _get_module()
    in_maps = _host_inputs(**inputs)
    res = run_bass_kernel_spmd(nc, in_maps, core_ids=list(range(NCORES)))
    outs = [np.asarray(r["outp"], dtype=np.float32) for r in res.results]
    out = np.stack([outs[0] + outs[1] + outs[2] + outs[3],
                    outs[4] + outs[5] + outs[6] + outs[7]])
    return out.astype(np.float32)



# revision 45
# speedup vs baseline: 1.0223x; 1.0069x over previous
"""Trainium2 Bass kernel for BailingMoeV2.5 linear attention layer.

Sharding: 8 cores = 2 batches x 4 head-groups. Core c handles batch c//4,
heads 4*(c%4) .. +4 (of 16). Each core computes its head-slice of
qkv/g projections, chunked ALiBi-decayed linear attention, group-RMSNorm,
sigmoid gate, and a partial dense output (its 512 rows of w_dense).
Host sums the 4 partial outputs per batch.

All matmuls in fp16 (1 cyc/row on PE, 10-bit mantissa), fp32 PSUM
accumulation, fp32 recurrent state master.

Layout strategy per core:
  - hiddenT (fp16, [d_in, s]) host-pre-transposed; projections of q,k,v
    token-major (hiddenT tiles stationary), g head-dim-major (W_g stationary).
  - q,k norm+rope in token-major (free-dim reductions), then PE-transposed
    per chunk into head-dim-major for attention matmuls.
  - attention output oT accumulates [e, i] in PSUM (intra + inter), group
    norm via PE ones-reduction + PE broadcast, gate in head-dim-major.
  - dense: ogT (fp16) stationary, w_dense moving -> token-major partial out.
"""

import math
from contextlib import ExitStack

import numpy as np

import concourse.mybir as mybir
import concourse.tile as tile
from concourse import bacc
from concourse.bass_utils import run_bass_kernel_spmd

dt = mybir.dt
F32 = dt.float32
F16 = dt.float16
AL = mybir.AluOpType
AF = mybir.ActivationFunctionType

# static model config
NH, HD, HID = 16, 128, 2048
ROT, HALF = 64, 32
EPS = 1e-6
THETA = 10000.0
LAYER_IDX, N_LAYERS = 1, 32
B, S = 2, 2048

DEBUG_TAPS = False
TAP_SET = ("q", "k", "v", "sg", "og", "st")

# kernel tiling config
NCORES = 8
NHL = 4            # heads per core
CH = 256           # internal chunk length (exact algebraic regrouping)
BLK = 512          # tokens per projection block
NBLK = S // BLK    # 4
SUBS = BLK // 128  # 4 s-subtiles per block
KT = HID // 128    # 16 d_in tiles
DOUT = NHL * HD    # 512 per tensor (q,k,v,g)


def _base_slopes(n):
    start = 2 ** (-(2 ** (-(math.log2(n) - 3))))
    return [start * (start ** i) for i in range(n)]


_SLOPE_ALL = -np.array(_base_slopes(NH), dtype=np.float64) * (
    1.0 - (LAYER_IDX - 1) / (N_LAYERS - 1) + 1e-5
)  # [NH] negative log-decay


def _build_module():
    nc = bacc.Bacc("TRN2", target_bir_lowering=False, debug=False,
                   num_devices=NCORES)

    f16in = lambda name, shape: nc.dram_tensor(
        name, shape, F16, kind="ExternalInput").ap()
    f32in = lambda name, shape: nc.dram_tensor(
        name, shape, F32, kind="ExternalInput").ap()

    d = {
        "hT": f16in("hT", [HID, S]),
        "wqkvg": f16in("wqkvg", [128, KT, 4 * DOUT]),
        "wd": f16in("wd", [128, NHL, HID]),
        "costab": f16in("costab", [128, S // 128, HALF]),
        "sintab": f16in("sintab", [128, S // 128, HALF]),
        "qnw": f16in("qnw", [128, HD]),
        "knw": f16in("knw", [128, HD]),
        "maskt": f16in("maskt", [128, NHL, 2, CH]),
        "identm": f16in("identm", [128, 128]),
        "onec": f16in("onec", [128, 1]),
        "oner": f16in("oner", [1, 128]),
        "qdecT": f16in("qdecT", [128, NHL, CH]),
        "kdec": f32in("kdec", [128, NHL, 2]),
        "lamc": f32in("lamc", [128, NHL]),
        "epsc": f32in("epsc", [1, 1]),
        "st0": f32in("st0", [128, NHL, HD]),
        "outp": nc.dram_tensor("outp", [S, HID], F16,
                               kind="ExternalOutput").ap(),
    }
    if DEBUG_TAPS:
        for nm, shape, dtp in [("dbg_q", [128, SUBS, DOUT], F16),
                               ("dbg_k", [128, SUBS, DOUT], F16),
                               ("dbg_v", [128, SUBS, DOUT], F16),
                               ("dbg_og", [128, NHL, BLK], F16),
                               ("dbg_st", [128, NHL, HD], F32),
                               ("dbg_sg", [128, NHL, BLK], F16)]:
            d[nm] = nc.dram_tensor(nm, shape, dtp,
                                   kind="ExternalOutput").ap()

    with tile.TileContext(nc) as tc, ExitStack() as ctx, \
            nc.allow_low_precision(reason="fp16 operands, fp32 accumulate"):
        _body(nc, tc, ctx, d)

    nc.compile()
    return nc


def _body(nc, tc, ctx, d):
    P = 128

    pool = lambda name, bufs: ctx.enter_context(
        tc.tile_pool(name=name, bufs=bufs))
    const = pool("const", 1)      # tables, masks, state, identity (~12k)
    wpool = pool("wpool", 1)      # 48k: resident weights (fp16)
    htpool = pool("ht", 34)       # 34k: hiddenT tiles, 2 blocks in flight
    qkvblk = pool("qkvblk", 8)    # 24k: q/k/v token-major, 2 blocks in flight
    sigp = pool("sigp", 2)        # 4k: sigmoid(g) head-dim-major (fp16)
    sqscp = pool("sqsc", 2)       # 2k: sumsq squares scratch (fp16)
    ropep = pool("ropep", 2)      # rope m1..m4 (fp16)
    ssp = pool("ssp", 8)          # ~0.5k: sumsq/rstd chains (fp32)
    trp = pool("trp", 2)          # 6k: qT/qdT/kT chunk tiles (fp16)
    stp = pool("stp", 3)          # 1.5k: masked scoresT (fp16)
    smallp = pool("smallp", 3)    # kdec-scaled v (fp16)
    stcp = pool("stcp", 2)        # 2k: fp16 state snapshot
    sqp = pool("sqp", 4)          # o squares + oS evictions
    rsp = pool("rsp", 2)          # gn rstd rows + broadcast tiles
    ogp = pool("ogp", 3)          # 12k: ogT fp16, 3 generations
    outsp = pool("outs", 3)       # 6k: dense output staging (fp32)

    psum = ctx.enter_context(tc.tile_pool(name="ps", bufs=8, space="PSUM"))
    psn = [0]

    def ps_tile(shape, dtype=F32):
        psn[0] += 1
        return psum.tile(shape, dtype, tag="ps", name=f"ps{psn[0]}")

    def load(pl, shape, dtype, name):
        t = pl.tile(shape, dtype, tag=name, name=name)
        nc.sync.dma_start(t[:], d[name])
        return t

    # per-k weight tiles + block-0 hT tiles, DMA-interleaved so the first
    # projection matmuls can start early instead of after the full 12MB
    w_tiles = []
    ht0_tiles = []
    for k in range(KT):
        wt = wpool.tile([P, 4 * DOUT], F16, tag=f"w{k}", name=f"w{k}")
        nc.sync.dma_start(wt[:], d["wqkvg"][:, k, :])
        w_tiles.append(wt)
        t = htpool.tile([P, BLK], F16, tag="ht")
        nc.sync.dma_start(t[:], d["hT"][k * 128:(k + 1) * 128, 0:BLK])
        ht0_tiles.append(t)
    wd_t = load(wpool, [P, NHL, HID], F16, "wd")
    cos_t = load(const, [P, S // 128, HALF], F16, "costab")
    sin_t = load(const, [P, S // 128, HALF], F16, "sintab")
    qnw_t = load(const, [P, HD], F16, "qnw")
    knw_t = load(const, [P, HD], F16, "knw")
    mask_t = load(const, [P, NHL, 2, CH], F16, "maskt")
    qdecT_t = load(const, [P, NHL, CH], F16, "qdecT")
    kdec_t = load(const, [P, NHL, 2], F32, "kdec")
    lamc_t = load(const, [P, NHL], F32, "lamc")
    eps_t = load(const, [1, 1], F32, "epsc")
    state_t = load(const, [P, NHL, HD], F32, "st0")
    ident = load(const, [P, P], F16, "identm")
    ones_col = load(const, [P, 1], F16, "onec")
    ones_row = load(const, [1, P], F16, "oner")

    def emit_ht(nn):
        ts = []
        for k in range(KT):
            t = htpool.tile([P, BLK], F16, tag="ht")
            nc.sync.dma_start(t[:], d["hT"][k * 128:(k + 1) * 128,
                                            nn * BLK:(nn + 1) * BLK])
            ts.append(t)
        return ts

    def make_proj(ht_tiles):
        """Allocate a block's q/k/v tiles; return 12 emit-closures (one PSUM
        accumulation group each: 16 matmuls + eviction + q/k sumsq)."""
        q_blk = [qkvblk.tile([P, DOUT], F16, tag="blk0", name="qb") for _ in range(SUBS)]
        k_blk = [qkvblk.tile([P, DOUT], F16, tag="blk1", name="kb") for _ in range(SUBS)]
        v_blk = [qkvblk.tile([P, DOUT], F16, tag="blk2", name="vb") for _ in range(SUBS)]
        ss_l = [ssp.tile([P, 8], F32, tag="ss", name="ss") for _ in range(SUBS)]
        groups = []
        for sub in range(SUBS):
            for ti, dest in enumerate((q_blk, k_blk, v_blk)):
                def grp(sub=sub, ti=ti, dest=dest, ss_t=ss_l[sub],
                        ht=ht_tiles):
                    ps = ps_tile([P, DOUT])
                    for k in range(KT):
                        nc.tensor.matmul(
                            ps[:], ht[k][:, sub * 128:(sub + 1) * 128],
                            w_tiles[k][:, ti * DOUT:(ti + 1) * DOUT],
                            start=(k == 0), stop=(k == KT - 1))
                    sb = dest[sub]
                    nc.scalar.copy(sb[:], ps[:])
                    if ti < 2:
                        sqs = sqscp.tile([P, DOUT], F16, tag="sqscratch")
                        for h in range(NHL):
                            nc.vector.tensor_mul(
                                sqs[:, h * HD:(h + 1) * HD],
                                sb[:, h * HD:(h + 1) * HD],
                                sb[:, h * HD:(h + 1) * HD])
                            nc.vector.tensor_reduce(
                                ss_t[:, ti * 4 + h:ti * 4 + h + 1],
                                sqs[:, h * HD:(h + 1) * HD],
                                mybir.AxisListType.X, AL.add)
                groups.append(grp)
        return groups, (q_blk, k_blk, v_blk, ss_l)

    def emit_norm_rope(n, blk_state):
        q_blk, k_blk, v_blk, ss_l = blk_state
        for sub in range(SUBS):
            gs = n * SUBS + sub
            rstd_t = ssp.tile([P, 8], F32, tag="rstd")
            nc.vector.tensor_scalar(rstd_t[:], ss_l[sub][:], 1.0 / HD, EPS,
                                    AL.mult, AL.add)
            nc.vector.reciprocal(rstd_t[:], rstd_t[:])
            nc.scalar.activation(rstd_t[:], rstd_t[:], AF.Sqrt)
            for ti, (blk, nw_t) in enumerate(((q_blk, qnw_t), (k_blk, knw_t))):
                x = blk[sub]
                for h in range(NHL):
                    nc.vector.scalar_tensor_tensor(
                        x[:, h * HD:(h + 1) * HD], x[:, h * HD:(h + 1) * HD],
                        rstd_t[:, ti * 4 + h:ti * 4 + h + 1], nw_t[:],
                        AL.mult, AL.mult)
                x3 = x.rearrange("p (h d) -> p h d", h=NHL)
                x1, x2 = x3[:, :, 0:HALF], x3[:, :, HALF:ROT]
                cosb = cos_t[:, gs, :].unsqueeze(1).to_broadcast(
                    (P, NHL, HALF))
                sinb = sin_t[:, gs, :].unsqueeze(1).to_broadcast(
                    (P, NHL, HALF))
                m1 = ropep.tile([P, NHL, HALF], F16, tag="m1")
                m2 = ropep.tile([P, NHL, HALF], F16, tag="m2")
                m3 = ropep.tile([P, NHL, HALF], F16, tag="m3")
                m4 = ropep.tile([P, NHL, HALF], F16, tag="m4")
                nc.vector.tensor_mul(m1[:], x1, cosb)
                nc.vector.tensor_mul(m2[:], x2, sinb)
                nc.vector.tensor_mul(m3[:], x2, cosb)
                nc.vector.tensor_mul(m4[:], x1, sinb)
                nc.vector.tensor_sub(x1, m1[:], m2[:])
                nc.vector.tensor_add(x2, m3[:], m4[:])

    def dense_group(nn, ogT, sub, dm):
        ps = ps_tile([P, BLK])
        for kk in range(NHL):
            nc.tensor.matmul(
                ps[:], ogT[:, kk, sub * 128:(sub + 1) * 128],
                wd_t[:, kk, dm * BLK:(dm + 1) * BLK],
                start=(kk == 0), stop=(kk == NHL - 1))
        osb = outsp.tile([P, BLK], F16, tag="osb")
        nc.scalar.copy(osb[:], ps[:])
        eng = nc.sync if dm % 2 == 0 else nc.scalar
        eng.dma_start(
            d["outp"][nn * BLK + sub * 128:nn * BLK + (sub + 1) * 128,
                      dm * BLK:(dm + 1) * BLK], osb[:])

    # prologue: block 0 projections emitted directly
    groups0, cur = make_proj(ht0_tiles)
    for g in groups0:
        g()
    ht_cur = ht0_tiles

    filler = []

    def drain(k):
        for _ in range(min(k, len(filler))):
            filler.pop(0)()

    for n in range(NBLK):
        q_blk, k_blk, v_blk, ss_l = cur
        drain(len(filler))  # leftover dense from previous block

        emit_norm_rope(n, cur)

        # g projection emitted lazily (interleaved into attention)
        sig_blk = sigp.tile([P, NHL, BLK], F16, tag="sig")

        def g_proj_group(mg, ht=ht_cur, sig=sig_blk):
            ps = ps_tile([P, BLK])
            for k in range(KT):
                nc.tensor.matmul(
                    ps[:],
                    w_tiles[k][:, 3 * DOUT + mg * 128:3 * DOUT + (mg + 1) * 128],
                    ht[k][:], start=(k == 0), stop=(k == KT - 1))
            nc.scalar.activation(sig[:, mg, :], ps[:], AF.Sigmoid)

        filler.extend([lambda mg=mg: g_proj_group(mg) for mg in range(NHL)])

        # enqueue next block's hT loads + projection groups as filler
        if n + 1 < NBLK:
            ht_nxt = emit_ht(n + 1)
            groups, nxt = make_proj(ht_nxt)
            filler.extend(groups)
        else:
            nxt = None
        drain(2)

        if DEBUG_TAPS and n == 0:
            for sub in range(SUBS):
                if "q" in TAP_SET:
                    nc.sync.dma_start(d["dbg_q"][:, sub, :], q_blk[sub][:])
                if "k" in TAP_SET:
                    nc.sync.dma_start(d["dbg_k"][:, sub, :], k_blk[sub][:])
                if "v" in TAP_SET:
                    nc.sync.dma_start(d["dbg_v"][:, sub, :], v_blk[sub][:])

        # ---- attention: 2 chunks of 256, phase-wise over heads ----
        ogT_blk = ogp.tile([P, NHL, BLK], F16, tag="ogT")
        for cc in range(2):
            subs = (2 * cc, 2 * cc + 1)

            # fp16 snapshot of state (pre-update) + decay-scaled copies,
            # batched across heads via free-dim broadcast of the decay cols
            st_c = stcp.tile([P, NHL, HD], F16, tag="stc")
            nc.vector.tensor_copy(st_c[:], state_t[:])
            vd_c = []
            for si, sub in enumerate(subs):
                vd = smallp.tile([P, NHL, HD], F16, tag="vd")
                nc.vector.tensor_mul(
                    vd[:], v_blk[sub][:].rearrange("p (h e) -> p h e", h=NHL),
                    kdec_t[:, :, si].unsqueeze(2).to_broadcast((P, NHL, HD)))
                vd_c.append(vd)

            # transposes: per (tensor, si) 4 head-transposes share one PSUM
            # tile and one [128, 512] eviction
            qT_all = trp.tile([P, NHL, CH], F16, tag="qT")
            qdT_all = trp.tile([P, NHL, CH], F16, tag="qdT")
            kT_all = trp.tile([P, NHL, CH], F16, tag="kT")
            for si, sub in enumerate(subs):
                sl = slice(si * 128, (si + 1) * 128)
                pt = ps_tile([P, NHL * P], F16)
                for h in range(NHL):
                    nc.tensor.transpose(
                        pt[:, h * 128:(h + 1) * 128],
                        q_blk[sub][:, h * HD:(h + 1) * HD], ident[:])
                nc.scalar.copy(qT_all[:, :, sl],
                               pt[:].rearrange("p (h x) -> p h x", h=NHL))
                drain(1)
                pt3 = ps_tile([P, NHL * P], F16)
                for h in range(NHL):
                    nc.tensor.transpose(
                        pt3[:, h * 128:(h + 1) * 128],
                        k_blk[sub][:, h * HD:(h + 1) * HD], ident[:])
                nc.scalar.copy(kT_all[:, :, sl],
                               pt3[:].rearrange("p (h x) -> p h x", h=NHL))
                drain(1)
            nc.vector.tensor_mul(qdT_all[:], qT_all[:], qdecT_t[:])
            qT = [qT_all[:, h, :] for h in range(NHL)]
            qdT = [qdT_all[:, h, :] for h in range(NHL)]
            kT = [kT_all[:, h, :] for h in range(NHL)]

            # scores for all heads, masked
            sT = []
            for h in range(NHL):
                sTh = []
                for si in range(2):
                    ps = ps_tile([P, CH])
                    nc.tensor.matmul(ps[:], kT[h][:, si * 128:(si + 1) * 128],
                                     qT[h], start=True, stop=True)
                    st = stp.tile([P, CH], F16, tag="sT")
                    nc.vector.tensor_mul(st[:], ps[:], mask_t[:, h, si, :])
                    sTh.append(st)
                sT.append(sTh)
                if h % 2 == 1:
                    drain(1)

            # o (intra + inter) and state delta + update per head
            sq_tiles = []
            o_sb_l = []
            for h in range(NHL):
                o_ps = ps_tile([P, CH])
                for si, sub in enumerate(subs):
                    nc.tensor.matmul(
                        o_ps[:], v_blk[sub][:, h * HD:(h + 1) * HD],
                        sT[h][si][:], start=(si == 0), stop=False)
                nc.tensor.matmul(o_ps[:], st_c[:, h, :], qdT[h],
                                 start=False, stop=True)

                dl_ps = ps_tile([P, HD])
                for si, sub in enumerate(subs):
                    nc.tensor.matmul(
                        dl_ps[:], k_blk[sub][:, h * HD:(h + 1) * HD],
                        vd_c[si][:, h, :], start=(si == 0), stop=(si == 1))
                nc.vector.scalar_tensor_tensor(
                    state_t[:, h, :], state_t[:, h, :],
                    lamc_t[:, h:h + 1], dl_ps[:], AL.mult, AL.add)

                # scale 1/64 before squaring: o can reach ~1e3 for
                # weak-decay heads and o^2 would overflow fp16
                sq = sqp.tile([P, CH], F16, tag="sq")
                nc.scalar.activation(sq[:], o_ps[:], AF.Square,
                                     scale=1.0 / 64.0)
                # evict o to SBUF so the PSUM slot frees without waiting
                # for the group-norm chain
                oS = sqp.tile([P, CH], F16, tag="oS")
                nc.scalar.copy(oS[:], o_ps[:])
                sq_tiles.append(sq)
                o_sb_l.append(oS)
                if h % 2 == 1:
                    # start group h//2's norm chain immediately so its
                    # latency overlaps the remaining o/dl matmuls
                    g = h // 2
                    gss = ps_tile([1, CH])
                    for hh in (2 * g, 2 * g + 1):
                        nc.tensor.matmul(gss[:], ones_col[:],
                                         sq_tiles[hh][:],
                                         start=(hh == 2 * g),
                                         stop=(hh == 2 * g + 1))
                    rrow = rsp.tile([1, CH], F16, tag="rrow")
                    nc.scalar.activation(rrow[:], gss[:], AF.Sqrt,
                                         bias=eps_t[:],
                                         scale=4096.0 / (2 * HD))
                    bc = rsp.tile([P, CH], F16, tag="bc")
                    nc.gpsimd.partition_broadcast(bc[:], rrow[:])
                    nc.vector.reciprocal(bc[:], bc[:])
                    nc.vector.tensor_mul(
                        sig_blk[:, 2 * g:2 * g + 2, cc * CH:(cc + 1) * CH],
                        sig_blk[:, 2 * g:2 * g + 2, cc * CH:(cc + 1) * CH],
                        bc[:].unsqueeze(1).to_broadcast((P, 2, CH)))
                    for hh in (2 * g, 2 * g + 1):
                        nc.vector.tensor_mul(
                            ogT_blk[:, hh, cc * CH:(cc + 1) * CH],
                            o_sb_l[hh][:],
                            sig_blk[:, hh, cc * CH:(cc + 1) * CH])
                    drain(1)

            for sub in (2 * cc, 2 * cc + 1):
                for dm in range(HID // BLK):
                    filler.append(
                        lambda nn=n, og=ogT_blk, s=sub, m=dm:
                        dense_group(nn, og, s, m))
            drain(2)

        if DEBUG_TAPS and n == 0:
            if "sg" in TAP_SET:
                nc.sync.dma_start(d["dbg_sg"], sig_blk[:])
            if "og" in TAP_SET:
                nc.sync.dma_start(d["dbg_og"], ogT_blk[:])
            if "st" in TAP_SET:
                st_dump = outsp.tile([P, NHL, HD], F32, tag="stdump")
                nc.vector.tensor_copy(st_dump[:], state_t[:])
                nc.sync.dma_start(d["dbg_st"], st_dump[:])

        cur = nxt
        ht_cur = ht_nxt if n + 1 < NBLK else None

    drain(len(filler))


_NC_CACHE = None


def _get_module():
    global _NC_CACHE
    if _NC_CACHE is None:
        _NC_CACHE = _build_module()
    return _NC_CACHE


def _host_inputs(positions, hidden_states, recurrent_state, w_qkv, w_g,
                 w_dense, q_norm_w, k_norm_w, g_norm_w):
    """Build the 8 per-core input dicts."""
    F16NP = np.float16
    positions = np.asarray(positions)
    hidden_states = np.asarray(hidden_states, dtype=np.float32)
    recurrent_state = np.asarray(recurrent_state, dtype=np.float32)
    w_qkv = np.asarray(w_qkv, dtype=np.float32)
    w_g = np.asarray(w_g, dtype=np.float32)
    w_dense = np.asarray(w_dense, dtype=np.float32)
    q_norm_w = np.asarray(q_norm_w, dtype=np.float32)
    k_norm_w = np.asarray(k_norm_w, dtype=np.float32)
    g_norm_w = np.asarray(g_norm_w, dtype=np.float32)

    # rope tables from positions: [S, HALF] -> [128, S//128, HALF]
    inv_freq = 1.0 / (THETA ** (np.arange(HALF, dtype=np.float64) / HALF))
    ang = positions.astype(np.float64)[:, None] * inv_freq[None, :]
    cos = np.cos(ang).reshape(S // 128, 128, HALF).transpose(1, 0, 2)
    sin = np.sin(ang).reshape(S // 128, 128, HALF).transpose(1, 0, 2)
    cos = np.ascontiguousarray(cos.astype(F16NP))
    sin = np.ascontiguousarray(sin.astype(F16NP))

    qnw = np.ascontiguousarray(np.tile(q_norm_w[None, :], (128, 1))
                               .astype(F16NP))
    knw = np.ascontiguousarray(np.tile(k_norm_w[None, :], (128, 1))
                               .astype(F16NP))

    i_idx = np.arange(CH, dtype=np.float64)
    j_idx = np.arange(CH, dtype=np.float64)

    in_maps = []
    for core in range(NCORES):
        b = core // 4
        h0 = (core % 4) * NHL  # global head offset
        cs, ce = h0 * HD, (h0 + NHL) * HD

        hT = np.ascontiguousarray(hidden_states[b].T.astype(F16NP))

        w_all = np.concatenate(
            [w_qkv[:, cs:ce], w_qkv[:, NH * HD + cs:NH * HD + ce],
             w_qkv[:, 2 * NH * HD + cs:2 * NH * HD + ce],
             w_g[:, cs:ce]], axis=1)  # [HID, 4*DOUT]
        wqkvg = np.ascontiguousarray(
            w_all.reshape(KT, 128, 4 * DOUT).transpose(1, 0, 2).astype(F16NP))

        wd = (w_dense[cs:ce, :] * g_norm_w[cs:ce, None]).astype(F16NP)
        wd = np.ascontiguousarray(wd.reshape(NHL, 128, HID).transpose(1, 0, 2))

        loglam = _SLOPE_ALL[h0:h0 + NHL]  # [NHL] negative
        # maskt[p, h, jsub, i] = lam^(i - (jsub*128+p)) for i >= j else 0
        mm = np.where(i_idx[None, None, :] >= j_idx[None, :, None],
                      np.exp(loglam[:, None, None]
                             * (i_idx[None, None, :] - j_idx[None, :, None])),
                      0.0)  # [NHL, j, i]
        maskt = np.ascontiguousarray(
            mm.reshape(NHL, 2, 128, CH).transpose(2, 0, 1, 3).astype(F16NP))

        qdecT = np.exp(loglam[:, None] * (i_idx[None, :] + 1.0))  # [NHL, CH]
        qdecT = np.ascontiguousarray(
            np.broadcast_to(qdecT[None, :, :], (128, NHL, CH))
            .astype(F16NP))
        kdec = np.exp(loglam[:, None] * (CH - 1.0 - j_idx[None, :]))
        kdec = np.ascontiguousarray(
            kdec.reshape(NHL, 2, 128).transpose(2, 0, 1).astype(np.float32))
        lamc = np.tile(np.exp(loglam * CH).astype(np.float32)[None, :],
                       (128, 1))
        lamc = np.ascontiguousarray(lamc)

        st0 = np.ascontiguousarray(
            recurrent_state[b, h0:h0 + NHL].transpose(1, 0, 2))  # [d, h, e]

        in_maps.append({
            "identm": np.eye(128, dtype=F16NP),
            "onec": np.ones((128, 1), F16NP),
            "oner": np.ones((1, 128), F16NP),
            "hT": hT, "wqkvg": wqkvg, "wd": wd,
            "costab": cos, "sintab": sin, "qnw": qnw, "knw": knw,
            "maskt": maskt, "qdecT": qdecT, "kdec": kdec, "lamc": lamc,
            "epsc": np.full((1, 1), EPS, np.float32),
            "st0": st0.astype(np.float32),
        })
    return in_maps


def kernel(**inputs):
    nc = _get_module()
    in_maps = _host_inputs(**inputs)
    res = run_bass_kernel_spmd(nc, in_maps, core_ids=list(range(NCORES)))
    outs = [np.asarray(r["outp"], dtype=np.float32) for r in res.results]
    out = np.stack([outs[0] + outs[1] + outs[2] + outs[3],
                    outs[4] + outs[5] + outs[6] + outs[7]])
    return out.astype(np.float32)

